# revision 11
# baseline (speedup 1.0000x reference)
"""Embedding-bag kernel for Trainium2, 8 NeuronCores — v3.

v2 + three padding/overlap optimizations:
- Per (core, table) the 512 batch rows are sorted by chunk-0 token count
  before being split into 128-row blocks. With 2 chunks, c1 = SEQ - c0, so
  one sort tightens both chunks' per-block maxima (padding ~24% -> ~5%).
  Outputs come back row-permuted and are un-permuted on the host.
- K is the exact per-block max (no round-to-8): each job emits 8-column
  gathers plus one remainder gather (num_idxs = cols*128 <= 1024).
- The wrapped index tensor is loaded in two DMAs so gathers start early.
"""

import sys

if "/opt/trn_rl_repo" not in sys.path:
    sys.path.insert(0, "/opt/trn_rl_repo")

from contextlib import ExitStack

import numpy as np

import concourse.bacc as bacc
import concourse.bass as bass
import concourse.mybir as mybir
from concourse import library_config
from concourse.bass_utils import run_bass_kernel_spmd

N_CORES = 8
P = 128
VOCAB = 100000
SEQ = 200
DIM = 64
BATCH = 4096

N_CHUNKS = 2
CHUNK = VOCAB // N_CHUNKS          # 50000 (signed int16 indexing)
CHUNK_ROWS = CHUNK + 1             # + zero pad row
BASE_SHIFT = 32768                 # in_ap base shifted this many rows in
PAD_IDX = CHUNK - BASE_SHIFT       # local index of the zero row (positive)
GMAX_COLS = 8                      # max dest columns per gather (1024 idxs)
NBUF = 4
NQ = 4


def _gather_plan(kj):
    """Split kj columns into gathers of <=8 columns."""
    sizes = [GMAX_COLS] * (kj // GMAX_COLS)
    if kj % GMAX_COLS:
        sizes.append(kj % GMAX_COLS)
    return sizes


def build_nc(K, n_blocks, idx_cols, split_col):
    """K: [2, N_CHUNKS, n_blocks] exact max counts (identical across cores).
    idx_cols: total int16 columns of gidx. split_col: boundary of the first
    idx DMA (jobs whose columns start past it wait for the second DMA)."""
    kmax = int(K.max())

    nc = bacc.Bacc("TRN2", debug=False, num_swdge_queues=NQ)

    emb_cat = nc.dram_tensor(
        "emb_cat", [2 * N_CHUNKS * CHUNK_ROWS, DIM], mybir.dt.float32,
        kind="ExternalInput",
    )
    gidx = nc.dram_tensor("gidx", [P, idx_cols], mybir.dt.int16, kind="ExternalInput")
    out_pri = nc.dram_tensor("out_pri", [n_blocks * P, DIM], mybir.dt.float32, kind="ExternalOutput")
    out_sec = nc.dram_tensor("out_sec", [n_blocks * P, DIM], mybir.dt.float32, kind="ExternalOutput")
    outs = (out_pri, out_sec)

    jobs = [(t, b, k) for t in range(2) for b in range(n_blocks) for k in range(N_CHUNKS)]

    with (
        nc.Block() as _block,
        nc.sbuf_tensor("gidx_sb", [P, idx_cols], mybir.dt.int16) as gidx_sb,
        nc.semaphore("io") as io,
        ExitStack() as stack,
    ):
        slots = [
            stack.enter_context(
                nc.sbuf_tensor(f"slot{i}", [P, kmax * DIM], mybir.dt.float32)
            )
            for i in range(NBUF)
        ]
        accs = [
            stack.enter_context(
                nc.sbuf_tensor(f"acc{t}_{b}", [P, DIM], mybir.dt.float32)
            )
            for t in range(2)
            for b in range(n_blocks)
        ]
        tmp = stack.enter_context(nc.sbuf_tensor("tmp", [P, DIM], mybir.dt.float32))
        done = [
            [stack.enter_context(nc.semaphore(f"done{i}_{q}")) for q in range(NQ)]
            for i in range(NBUF)
        ]
        free = [stack.enter_context(nc.semaphore(f"free{i}")) for i in range(NBUF)]
        oready = stack.enter_context(nc.semaphore("oready"))
        vchain = stack.enter_context(nc.semaphore("vchain"))

        # ---- sync engine: two-stage index load so gathers start early
        nc.sync.dma_start(gidx_sb[:, :split_col], gidx[:, :split_col]).then_inc(io, 16)
        nc.sync.dma_start(gidx_sb[:, split_col:], gidx[:, split_col:]).then_inc(io, 16)

        # ---- gpsimd: all gathers
        nc.gpsimd.load_library(library_config.mlp)
        nc.gpsimd.wait_ge(io, 16)
        waited_full = False
        gq = 0            # queue rotation counter
        icol = 0          # running int16 column offset into gidx_sb
        done_target = [[0] * NQ for _ in range(NBUF)]
        for j, (t, b, k) in enumerate(jobs):
            slot = j % NBUF
            if j >= NBUF:
                nc.gpsimd.wait_ge(free[slot], j // NBUF)
            kj = int(K[t, k, b])
            base = (t * N_CHUNKS + k) * CHUNK_ROWS + BASE_SHIFT
            src = emb_cat[base:(t * N_CHUNKS + k + 1) * CHUNK_ROWS, :]
            g3 = slots[slot][:].rearrange("p (c d) -> p c d", d=DIM)
            col = 0
            for size in _gather_plan(kj):
                nidx = size * P
                ic = nidx // 16
                if not waited_full and icol + ic > split_col:
                    nc.gpsimd.wait_ge(io, 32)
                    waited_full = True
                q = gq % NQ
                nc.gpsimd.dma_gather(
                    g3[:, col:col + size, :],
                    src,
                    gidx_sb[:, icol:icol + ic],
                    nidx,
                    nidx,
                    DIM,
                    queue_num=q,
                ).then_inc(done[slot][q], 16)
                done_target[slot][q] += 16
                gq += 1
                icol += ic
                col += size
            jobs[j] = (t, b, k, slot, tuple(done_target[slot]), kj)

        # ---- vector: tree-reduce, accumulate chunks, recycle slots
        vc = 0
        for j, (t, b, k, slot, tgts, kj) in enumerate(jobs):
            for q in range(NQ):
                if tgts[q]:
                    nc.vector.wait_ge(done[slot][q], tgts[q])
            g = slots[slot]
            # one contiguous halving pass (1 elem/cycle) then a strided
            # reduce (0.5 elem/cycle) over the remaining columns
            n = kj
            if n > 1:
                h = n // 2
                nc.vector.tensor_add(
                    out=g[:, : h * DIM],
                    in0=g[:, : h * DIM],
                    in1=g[:, (n - h) * DIM : n * DIM],
                )
                n -= h
            gv = g[:].rearrange("p (c d) -> p d c", d=DIM)[:, :, :n]
            acc = accs[t * n_blocks + b]
            red_out = acc if k == 0 else tmp
            nc.vector.tensor_reduce(
                out=red_out[:], in_=gv, axis=mybir.AxisListType.X,
                op=mybir.AluOpType.add,
            )
            if k != 0:
                nc.vector.tensor_add(out=acc[:], in0=acc[:], in1=tmp[:])
            nc.vector.tensor_copy(out=g[:, :4], in_=g[:, :4]).then_inc(vchain, 1)
            vc += 1
            nc.vector.wait_ge(vchain, vc)
            nc.vector.sem_inc(free[slot], 1)
            if k == N_CHUNKS - 1:
                nc.vector.sem_inc(oready, 1)

        # ---- sync engine: write outputs as accs complete
        m = 0
        for t in range(2):
            for b in range(n_blocks):
                m += 1
                nc.sync.wait_ge(oready, m)
                nc.sync.dma_start(
                    out=outs[t][b * P:(b + 1) * P, :],
                    in_=accs[t * n_blocks + b][:],
                ).then_inc(io, 16)
        nc.sync.wait_ge(io, 32 + m * 16)

    nc.compile()
    return nc


def _pack_core(idx_sorted, K, n_blocks):
    """idx_sorted: [2, bc, SEQ] row-sorted core indices. Returns gidx."""
    streams = []
    for t in range(2):
        for b in range(n_blocks):
            rows = idx_sorted[t][b * P:(b + 1) * P]
            for k in range(N_CHUNKS):
                kj = int(K[t, k, b])
                mask = (rows // CHUNK) == k
                local = (rows - k * CHUNK - BASE_SHIFT).astype(np.int64)
                order = np.argsort(~mask, axis=1, kind="stable")
                sortloc = np.take_along_axis(local, order, axis=1)
                cnt = mask.sum(axis=1)
                pad_cols = max(kj - SEQ, 0)
                if pad_cols:
                    sortloc = np.concatenate(
                        [sortloc, np.zeros((P, pad_cols), np.int64)], axis=1
                    )
                sel = sortloc[:, :kj]
                sel = np.where(np.arange(kj)[None, :] < cnt[:, None], sel, PAD_IDX)
                # Every gather's final stream slot (lane 127, last column of
                # the gather) must be >= 0: ucode trims trailing negatives.
                row127 = sel[127].copy()
                lasts = []
                c = 0
                for size in _gather_plan(kj):
                    c += size
                    lasts.append(c - 1)
                lastset = set(lasts)
                for last in lasts:
                    if row127[last] < 0:
                        cand = [jj for jj in range(kj)
                                if row127[jj] >= 0 and jj not in lastset]
                        assert cand, "no non-negative index for lane 127"
                        jj = cand[0]
                        row127[last], row127[jj] = row127[jj], row127[last]
                sel[127] = row127
                # column-major stream, split per gather
                c = 0
                for size in _gather_plan(kj):
                    streams.append(sel[:, c:c + size].T.ravel())
                    c += size
    s = np.concatenate(streams).astype(np.int16)
    wrapped = s.reshape(-1, 16).T
    return np.tile(wrapped, (8, 1)).copy()


def kernel(inputs_pri, inputs_sec, emb_pri, emb_sec, _trace=False, _trace_kwargs=None):
    inputs_pri = np.ascontiguousarray(np.asarray(inputs_pri, dtype=np.int32))
    inputs_sec = np.ascontiguousarray(np.asarray(inputs_sec, dtype=np.int32))
    emb_pri = np.ascontiguousarray(np.asarray(emb_pri, dtype=np.float32))
    emb_sec = np.ascontiguousarray(np.asarray(emb_sec, dtype=np.float32))

    batch = inputs_pri.shape[0]
    bc = batch // N_CORES
    n_blocks = bc // P

    emb_cat = np.zeros((2, N_CHUNKS, CHUNK_ROWS, DIM), np.float32)
    for t, emb in enumerate((emb_pri, emb_sec)):
        for k in range(N_CHUNKS):
            emb_cat[t, k, :CHUNK] = emb[k * CHUNK:(k + 1) * CHUNK]
    emb_cat = np.ascontiguousarray(emb_cat.reshape(2 * N_CHUNKS * CHUNK_ROWS, DIM))

    # sort each core's rows by chunk-0 count (c1 = SEQ - c0 sorts with it)
    sorted_rows = []   # per core: [2][bc, SEQ]
    orders = []        # per core: [2][bc]
    K = np.zeros((2, N_CHUNKS, n_blocks), np.int64)
    for c in range(N_CORES):
        rows_c, ords_c = [], []
        for t, full in enumerate((inputs_pri, inputs_sec)):
            rows = full[c * bc:(c + 1) * bc]
            c0 = ((rows // CHUNK) == 0).sum(axis=1)
            order = np.argsort(c0, kind="stable")
            srt = rows[order]
            rows_c.append(srt)
            ords_c.append(order)
            c0s = c0[order]
            for b in range(n_blocks):
                blk = c0s[b * P:(b + 1) * P]
                K[t, 0, b] = max(K[t, 0, b], blk.max())
                K[t, 1, b] = max(K[t, 1, b], SEQ - blk.min())
        sorted_rows.append(rows_c)
        orders.append(ords_c)
    K = np.maximum(K, 1)

    total_cols = int(K.sum())
    idx_cols = total_cols * P // 16
    # first DMA covers just job 0's columns so gathers start ASAP; the second
    # (bulk) DMA overlaps job 0's gathers
    split_col = max(64, (int(K[0, 0, 0]) * P // 16) // 64 * 64)
    split_col = min(split_col, idx_cols - 64)

    nc = build_nc(K, n_blocks, idx_cols, split_col)

    in_maps = []
    for c in range(N_CORES):
        gidx = _pack_core(sorted_rows[c], K, n_blocks)
        assert gidx.shape[1] == idx_cols
        in_maps.append({"emb_cat": emb_cat, "gidx": gidx})

    kwargs = {}
    if _trace:
        kwargs["trace"] = True
        if _trace_kwargs:
            kwargs.update(_trace_kwargs)
    res = run_bass_kernel_spmd(nc, in_maps, list(range(N_CORES)), **kwargs)
    outs = res.results
    out_pri = np.empty((batch, DIM), np.float32)
    out_sec = np.empty((batch, DIM), np.float32)
    for c in range(N_CORES):
        for t, out_full in enumerate((out_pri, out_sec)):
            res_c = outs[c]["out_pri" if t == 0 else "out_sec"]
            out_full[c * bc + orders[c][t]] = res_c
    if _trace:
        return (out_pri, out_sec), res
    return out_pri, out_sec
